# revision 54
# baseline (speedup 1.0000x reference)
"""RWKV time-mixing (C=4096) on 8 trn2 NeuronCores.

Strategy (tensor-parallel over channels, M=8 cores, S=C/M=512):
  - Core c owns channels sl = [c*512, (c+1)*512).
  - The kernel is HBM-bound: it streams the four weight matrices once, in
    bf16 (halves fp32 traffic; measured rel-err 3.3e-3 vs the 2e-2 budget;
    fp8-e4m3 for ow+y was measured at 3.4e-2 — over budget — so bf16 is
    the floor). All elementwise WKV math stays fp32, and the PE-reshape
    trick stays fp32r so kk keeps full precision into exp(). Total
    stream: 16 MB/core.
  - Phase 1: kk/vv/rr shards: kw[sl,:] @ xk etc. Weights are host-transposed
    so the contraction dim lands on SBUF partitions; the x-vector column is
    the (tiny) stationary operand, weight tiles stream as the moving operand
    at N=512 (1 cycle/row on the PE).
  - WKV recurrence: purely elementwise on the 512-channel shard, done in a
    [128, 4] layout (channel = j*128 + p).
  - Phase 2: partial out = ow[:, sl] @ (r*wkv): 4 k-tiles x 8 n-banks of
    matmuls into a [1, 4096] PSUM row; host sums the 8 per-core partials
    (the "all-reduce" of the column-sharded matvec).
  - new_state = x exactly (token shift), assembled on host.

Stream order is MATRIX-MAJOR (all of rw, then kw, then vw, then ow) so the
dependent chain drains while later weights stream: rr is complete early
(sigmoid runs there), kk next (the kk-only part of the WKV recurrence runs
there), vv third (the short vv tail + y = r*wkv run there, just before the
first ow chunk lands), and the phase-2 matmuls then chase the ow chunks as
they arrive. Only the last ow half-chunk's four matmuls + PSUM copies +
output DMA remain after the final weight byte.

DMA chunks are 1 MB (8 KB per-partition rows; a DMA-only probe measured
4 KB rows ~27% slower and, counterintuitively, 16/32 KB rows ~20% slower
still — 8 KB is the sweet spot). Only ow's last k-tile is split into
512 KB halves, where the arrival quantum sits on the critical tail. The
output DMAs ride the sync HWDGE ring (idle once the weight stream ends;
measured ~1 us faster than a SWDGE descriptor write on the tail).

The PE runs every matmul at ap_size cycles (216 ns at N=512) regardless of
the 1-wide stationary, so phase 1+2 carry ~28 us of mandatory PE time that
must hide under the ~44 us stream. The PE also stalls ~6-8 us in a p-state
transition after its first matmul, so a tiny f32 matmul fires the
transition at ~6 us and the stall completes as the first chunk lands.

k-index convention (phase 1): k = p*32 + n  (p = partition, n = k-tile id),
so W.T.reshape(128, 32, 512) puts k-tile n at [:, n, :] with contraction on
partitions, and x.reshape(128, 32) column n is the matching stationary vec.

The [1,512] -> [128,4] reshape of the phase-1 results runs on the PE: a
matmul whose stationary operand is a zero-padded [128,128] slice with the
data in partition 0 and whose moving operand is the unit vector e0 emits
the row slice as a [128,2] PSUM column pair (channel = j*128 + p). Phase 2
contracts k-tile tt=j over rows [j*128,(j+1)*128) of ow[:, sl].T, matching
that layout.
"""

import ml_dtypes
import numpy as np

import concourse.bass as bass
import concourse.mybir as mybir
import concourse.tile as tile
from concourse import bacc, bass_utils

C = 4096
NCORES = 8
S = C // NCORES          # 512 channels per core
P = 128
KT = C // P              # 32 k-tiles in phase 1
KSUB = 8                 # k-tiles per 1 MB bf16 chunk
OW_KT = S // P           # 4 k-tiles in phase 2
OW_HALF = C // 2         # ow k-tile chunks split into 512 KB column halves
CHUNKS_STD = [(0, 8), (8, 8), (16, 8), (24, 8)]

F32 = mybir.dt.float32
F32R = mybir.dt.float32r
BF16 = mybir.dt.bfloat16
AF = mybir.ActivationFunctionType
NP_BF16 = ml_dtypes.bfloat16

# xvecs layout: [128, 96] = xk[0:32] | xv[32:64] | xr[64:96] (bf16)
# e0 (fp32r) is a separate [128, 2] input: unit vector [1,0,...,0] + a zero
# column (N=2 moving operand of the PE-reshape matmuls — fp32r matmuls
# require an even moving free size)
# svecs layout: [128, 20] = aa[0:4] | bb[4:8] | pp[8:12] | tf[12:16] | td[16:20]
XVECS_W = 3 * KT
SVECS_W = 5 * 4
OFF_XK, OFF_XV, OFF_XR = 0, KT, 2 * KT
OFF_AA, OFF_BB, OFF_PP, OFF_TF, OFF_TD = 0, 4, 8, 12, 16


def _build():
    nc = bacc.Bacc("TRN2", target_bir_lowering=False, debug=False,
                   num_devices=NCORES)

    xvecs_d = nc.dram_tensor("xvecs", [P, XVECS_W], BF16, kind="ExternalInput")
    e0_d = nc.dram_tensor("e0", [P, 2], F32R, kind="ExternalInput")
    svecs_d = nc.dram_tensor("svecs", [P, SVECS_W], F32, kind="ExternalInput")
    wr_d = nc.dram_tensor("wr", [P, KT, S], BF16, kind="ExternalInput")
    wk_d = nc.dram_tensor("wk", [P, KT, S], BF16, kind="ExternalInput")
    wv_d = nc.dram_tensor("wv", [P, KT, S], BF16, kind="ExternalInput")
    wo_d = nc.dram_tensor("wo", [OW_KT, P, C], BF16, kind="ExternalInput")

    partial_d = nc.dram_tensor("partial", [1, C], F32, kind="ExternalOutput")
    nst_d = nc.dram_tensor("nst", [P, 12], F32, kind="ExternalOutput")

    with tile.TileContext(nc) as tc:
        with (
            tc.tile_pool(name="const", bufs=1) as const,
            tc.tile_pool(name="w", bufs=12) as wpool,
            tc.tile_pool(name="ow", bufs=3) as opool,
            tc.tile_pool(name="owh", bufs=2) as ohpool,
            tc.tile_pool(name="small", bufs=1) as small,
        ):
            # z2 feeds the ramp-trigger matmul (emitted once the PSUM pool
            # opens below); its memset leads the gpsimd queue so the trigger
            # can fire as early as possible.
            z2 = small.tile([P, 2], F32)
            nc.gpsimd.memset(z2[:], 0.0)

            # Small INPUT DMAs ride the SWDGE (gpsimd) ring so the SP HWDGE
            # ring carries only weight traffic during the stream (keeps its
            # DMA-completion semaphore lanes clean); the output DMAs ride
            # the then-idle HWDGE ring instead. Emitted BEFORE the big
            # stage_z memset so the descriptor writes are not delayed.
            xvecs = const.tile([P, XVECS_W], BF16)
            nc.gpsimd.dma_start(xvecs[:], xvecs_d[:])
            # pinned weight tile for bridging/filler matmuls
            filler = const.tile([P, 1, S], BF16)
            nc.gpsimd.dma_start(filler[:], wr_d[:, 0:1, :])
            e0 = const.tile([P, 2], F32R)
            nc.gpsimd.dma_start(e0[:], e0_d[:])
            svecs = const.tile([P, SVECS_W], F32)
            nc.gpsimd.dma_start(svecs[:], svecs_d[:])

            # preload the ACT exp LUT off the critical path (the only ACT
            # table the kernel uses: sigmoid is computed via exp+reciprocal)
            warm = small.tile([1, 4], F32)
            nc.gpsimd.memset(warm[:], 0.0)
            warm2 = small.tile([1, 4], F32)
            nc.scalar.activation(warm2[:], warm[:], AF.Exp)

            # stage for the PE reshape: partition 0 carries the phase-1
            # results, rows 1-127 must be finite (they multiply e0's zeros).
            # f32r memset is invalid ISA, so zero an f32 twin and cast-copy.
            stage_z = small.tile([P, 3 * S], F32)
            nc.gpsimd.memset(stage_z[:], 0.0)
            stage = small.tile([P, 3 * S], F32R)
            nc.vector.tensor_copy(stage[:], stage_z[:])

            aa = svecs[:, OFF_AA:OFF_AA + 4]
            bb = svecs[:, OFF_BB:OFF_BB + 4]
            pp = svecs[:, OFF_PP:OFF_PP + 4]
            tf = svecs[:, OFF_TF:OFF_TF + 4]
            td = svecs[:, OFF_TD:OFF_TD + 4]

            def t4(name):
                return small.tile([P, 4], F32, name=name)

            nst = small.tile([P, 12], F32)
            na, nb_t, p2 = nst[:, 0:4], nst[:, 4:8], nst[:, 8:12]
            rkv = small.tile([P, 12], F32)
            rr128 = rkv[:, 0:4]
            kk = rkv[:, 4:8]
            vv = rkv[:, 8:12]

            # ---- phase 1 + overlapped recurrence ---------------------------
            wdrams = [wr_d, wk_d, wv_d]
            xoffs = [OFF_XR, OFF_XK, OFF_XV]
            # WKV temporaries shared between the kk-stage and the vv-stage.
            # y = r*(e1a*aa + e2a*vv)/b is regrouped as y = u + w*vv with
            # u = r*binv*e1a*aa and w = r*binv*e2a, both computable in the
            # kk-stage, so only TWO vector ops + the bf16 cast separate the
            # vv reshape from the phase-2 matmuls.
            r128 = small.tile([P, 4], F32, name="r128")
            e2b = t4("e2b")
            u_t = t4("u_t")
            w_t = t4("w_t")
            y = t4("y")
            y_r = small.tile([P, 4], BF16, name="y_r")

            with tc.tile_pool(name="ps1", bufs=1, space="PSUM") as ps1:
                psums = [ps1.tile([1, S], F32, name=f"ps_{i}") for i in range(3)]
                fl_ps = ps1.tile([1, S], F32, name="fl_ps")
                rs_ps = ps1.tile([P, 24], F32, name="rs_ps")

                # ramp trigger: the PE stalls ~6-8us in a p-state transition
                # after its FIRST matmul, then runs at half speed for ~3us.
                # Fire a tiny f32 matmul as early as possible (the z2 memset
                # is its only dependency) so the stall completes right as the
                # first weight chunk lands.
                zps = ps1.tile([2, 2], F32, name="zps")
                nc.tensor.matmul(zps[:], lhsT=z2[:], rhs=z2[:], start=True, stop=True)

                def fill_mm(n):
                    for i in range(n):
                        nc.tensor.matmul(
                            fl_ps[:],
                            lhsT=xvecs[:, 0:1],
                            rhs=filler[:, 0, :],
                            start=True,
                            stop=True,
                        )

                # bridge the PE from the ramp trigger to the first chunk
                fill_mm(2)
                for wi in range(3):
                    for (kt0, nkt) in CHUNKS_STD:
                        wt = wpool.tile([P, KSUB, S], BF16, tag="wchunk")
                        nc.sync.dma_start(
                            wt[:], wdrams[wi][:, kt0:kt0 + nkt, :])
                        for tl in range(nkt):
                            kt = kt0 + tl
                            nc.tensor.matmul(
                                psums[wi][:],
                                lhsT=xvecs[:, xoffs[wi] + kt:xoffs[wi] + kt + 1],
                                rhs=wt[:, tl, :],
                                start=(kt == 0),
                                stop=(kt == KT - 1),
                            )

                    # matrix wi fully reduced: transpose its [1,512] row into
                    # the [128,4] WKV layout while the next matrix streams
                    if wi == 0:
                        nc.scalar.copy(stage[0:1, 0:S], psums[0][:])
                    else:
                        nc.vector.tensor_copy(
                            stage[0:1, wi * S:(wi + 1) * S], psums[wi][:])
                    if wi == 2:
                        # the PE otherwise idles ~2us through the vv CAST
                        # roundtrip + vector tail and deramps to half clock,
                        # slowing the first ~14 phase-2 matmuls; these
                        # fillers keep it continuously busy through that
                        # window so phase 2 starts at full speed
                        fill_mm(3)
                    for j in range(OW_KT):
                        c2 = 2 * (wi * 4 + j)
                        nc.tensor.matmul(
                            rs_ps[:, c2:c2 + 2],
                            lhsT=stage[:, wi * S + j * P:wi * S + (j + 1) * P],
                            rhs=e0[:],
                            start=True,
                            stop=True,
                        )
                    if wi == 2:
                        fill_mm(4)
                    nc.vector.tensor_copy(
                        rkv[:, wi * 4:wi * 4 + 4],
                        rs_ps[:, 2 * wi * 4:2 * wi * 4 + 8:2])

                    if wi == 0:
                        # r = sigmoid(rr) = 1 / (1 + exp(-rr)) — exp only
                        er = t4("er")
                        nc.scalar.activation(er[:], rr128, AF.Exp, scale=-1.0)
                        rp1 = t4("rp1")
                        nc.vector.tensor_scalar_add(rp1[:], er[:], 1.0)
                        nc.vector.reciprocal(r128[:], rp1[:])
                    elif wi == 1:
                        # everything in the WKV recurrence that needs only kk
                        ww1 = t4("ww1")
                        nc.vector.tensor_add(ww1, tf, kk[:])
                        p1 = t4("p1")
                        nc.vector.tensor_max(p1, pp, ww1)
                        d1 = t4("d1")
                        nc.vector.tensor_sub(d1, pp, p1)
                        e1a = t4("e1a")
                        nc.scalar.activation(e1a, d1, AF.Exp)
                        d2 = t4("d2")
                        nc.vector.tensor_sub(d2, ww1, p1)
                        e2a = t4("e2a")
                        nc.scalar.activation(e2a[:], d2, AF.Exp)
                        acc_a = t4("acc_a")
                        nc.vector.tensor_mul(acc_a[:], e1a, aa)   # e1*aa
                        acc_b = t4("acc_b")
                        nc.vector.tensor_mul(acc_b, e1a, bb)
                        nc.vector.tensor_add(acc_b, acc_b, e2a[:])
                        binv = t4("binv")
                        nc.vector.reciprocal(binv[:], acc_b)
                        # pre-fold r/b into the two y terms: y = u + w*vv
                        rb = t4("rb")
                        nc.vector.tensor_mul(rb, r128[:], binv[:])
                        nc.vector.tensor_mul(u_t[:], acc_a[:], rb)
                        nc.vector.tensor_mul(w_t[:], e2a[:], rb)
                        # state update, kk-only part
                        ww2 = t4("ww2")
                        nc.vector.tensor_add(ww2, pp, td)
                        nc.vector.tensor_max(p2, ww2, kk[:])
                        d3 = t4("d3")
                        nc.vector.tensor_sub(d3, ww2, p2)
                        e1b = t4("e1b")
                        nc.scalar.activation(e1b, d3, AF.Exp)
                        d4 = t4("d4")
                        nc.vector.tensor_sub(d4, kk[:], p2)
                        nc.scalar.activation(e2b[:], d4, AF.Exp)
                        nc.vector.tensor_mul(na, e1b, aa)         # e1*aa
                        nc.vector.tensor_mul(nb_t, e1b, bb)
                        nc.vector.tensor_add(nb_t, nb_t, e2b[:])
                    else:
                        # minimal vv tail on the y-critical path: y = u + w*vv
                        # then the bf16 cast, BEFORE the off-path state update
                        nc.vector.tensor_mul(y[:], w_t[:], vv[:])
                        nc.vector.tensor_add(y[:], u_t[:], y[:])
                        nc.vector.tensor_copy(y_r[:], y[:])
                        tmp_b = t4("tmp_b")
                        nc.vector.tensor_mul(tmp_b, e2b[:], vv[:])
                        nc.vector.tensor_add(na, na, tmp_b)

                # ow weight DMAs: the in-order HWDGE ring plays them right
                # after the phase-1 weight DMAs, by which time y is ready
                # and the phase-2 matmuls chase the arriving chunks. k-tiles
                # 0-2 stream as full 1 MB chunks (8 KB per-partition rows —
                # a DMA-only probe measured 4 KB rows ~27% slower, costing
                # ~2.6 us over ow's 4 MB); only k-tile 3 is split into
                # 512 KB halves to keep the after-last-byte quantum small.
                ofull = []
                for tt in range(OW_KT - 1):
                    ot = opool.tile([P, C], BF16, tag="owfull")
                    nc.sync.dma_start(ot[:], wo_d[tt][:, :])
                    ofull.append(ot)
                ohalf = []
                for half in range(2):
                    ot = ohpool.tile([P, OW_HALF], BF16, tag="owhalf")
                    nc.sync.dma_start(
                        ot[:],
                        wo_d[OW_KT - 1][:, half * OW_HALF:(half + 1) * OW_HALF])
                    ohalf.append(ot)

            nc.gpsimd.dma_start(nst_d[:], nst[:])

            # ---- phase 2: partial = ow[:, sl] @ y ---------------------------
            with tc.tile_pool(name="ps2", bufs=1, space="PSUM") as ps2:
                ow_ps = ps2.tile([1, C], F32)
                out_sb = small.tile([1, C], F32)
                for tt in range(OW_KT):
                    for nb in range(C // 512):
                        if tt < OW_KT - 1:
                            rhs = ofull[tt][:, nb * 512:(nb + 1) * 512]
                        else:
                            half, col = nb // 4, nb % 4
                            rhs = ohalf[half][:, col * 512:(col + 1) * 512]
                        nc.tensor.matmul(
                            ow_ps[:, nb * 512:(nb + 1) * 512],
                            lhsT=y_r[:, tt:tt + 1],
                            rhs=rhs,
                            start=(tt == 0),
                            stop=(tt == OW_KT - 1),
                        )
                        if tt == OW_KT - 1:
                            # bank nb is complete; copy out while later banks
                            # are still accumulating
                            sl_ = slice(nb * 512, (nb + 1) * 512)
                            if nb % 2 == 0:
                                nc.vector.tensor_copy(
                                    out_sb[:, sl_], ow_ps[:, sl_])
                            else:
                                nc.scalar.copy(out_sb[:, sl_], ow_ps[:, sl_])
                            if nb == 3:
                                # first half of the output leaves while banks
                                # 4-7 are still accumulating
                                nc.sync.dma_start(
                                    partial_d[:, 0:C // 2], out_sb[:, 0:C // 2])

            nc.sync.dma_start(partial_d[:, C // 2:], out_sb[:, C // 2:])

    nc.compile()
    return nc


def _prep_in_maps(x, state, state_a, state_b, state_p,
                  time_mix_k, time_mix_v, time_mix_r,
                  time_first, time_decay, kw, vw, rw, ow):
    f = lambda a: np.ascontiguousarray(np.asarray(a), dtype=np.float32)
    x, state = f(x), f(state)
    tmk, tmv, tmr = f(time_mix_k), f(time_mix_v), f(time_mix_r)
    xk = (x * tmk + state * (1.0 - tmk)).reshape(P, KT)
    xv = (x * tmv + state * (1.0 - tmv)).reshape(P, KT)
    xr = (x * tmr + state * (1.0 - tmr)).reshape(P, KT)
    aa, bb, pp = f(state_a), f(state_b), f(state_p)
    tf, td = f(time_first), f(time_decay)
    kw, vw, rw, ow = f(kw), f(vw), f(rw), f(ow)

    xvecs = np.zeros((P, XVECS_W), dtype=np.float32)
    xvecs[:, OFF_XK:OFF_XK + KT] = xk
    xvecs[:, OFF_XV:OFF_XV + KT] = xv
    xvecs[:, OFF_XR:OFF_XR + KT] = xr
    xvecs = xvecs.astype(NP_BF16)

    e0 = np.zeros((P, 2), dtype=np.float32)
    e0[0, 0] = 1.0

    wb = lambda a: np.ascontiguousarray(a).astype(NP_BF16)

    # WKV-side [128, 4] layout: channel = j*128 + p
    pm = lambda v: np.ascontiguousarray(v.reshape(OW_KT, P).T)
    in_maps = []
    for c in range(NCORES):
        sl = slice(c * S, (c + 1) * S)
        svecs = np.empty((P, SVECS_W), dtype=np.float32)
        svecs[:, OFF_AA:OFF_AA + 4] = pm(aa[sl])
        svecs[:, OFF_BB:OFF_BB + 4] = pm(bb[sl])
        svecs[:, OFF_PP:OFF_PP + 4] = pm(pp[sl])
        svecs[:, OFF_TF:OFF_TF + 4] = pm(tf[sl])
        svecs[:, OFF_TD:OFF_TD + 4] = pm(td[sl])
        in_maps.append({
            "xvecs": xvecs,
            "e0": e0,
            "svecs": svecs,
            "wr": wb(rw[sl, :].T).reshape(P, KT, S),
            "wk": wb(kw[sl, :].T).reshape(P, KT, S),
            "wv": wb(vw[sl, :].T).reshape(P, KT, S),
            "wo": wb(ow[:, sl].T).reshape(OW_KT, P, C),
        })
    return in_maps, x


_NC_CACHE = None


def _run(inputs, trace=False):
    global _NC_CACHE
    if _NC_CACHE is None:
        _NC_CACHE = _build()
    nc = _NC_CACHE
    in_maps, x = _prep_in_maps(**inputs)
    res = bass_utils.run_bass_kernel_spmd(
        nc, in_maps, core_ids=list(range(NCORES)), trace=trace)

    out = np.zeros(C, dtype=np.float32)
    new_a = np.empty(C, dtype=np.float32)
    new_b = np.empty(C, dtype=np.float32)
    new_p = np.empty(C, dtype=np.float32)
    for c in range(NCORES):
        r = res.results[c]
        out += r["partial"].reshape(C)
        sl = slice(c * S, (c + 1) * S)
        nst = r["nst"]
        # [p, j] -> channel j*128 + p
        new_a[sl] = nst[:, 0:4].T.reshape(S)
        new_b[sl] = nst[:, 4:8].T.reshape(S)
        new_p[sl] = nst[:, 8:12].T.reshape(S)
    return (out, x.copy(), new_a, new_b, new_p), res


def kernel(**inputs):
    outs, _ = _run(inputs, trace=False)
    return outs


# revision 55
# speedup vs baseline: 1.0508x; 1.0508x over previous
"""RWKV time-mixing (C=4096) on 8 trn2 NeuronCores.

Strategy (tensor-parallel over channels, M=8 cores, S=C/M=512):
  - Core c owns channels sl = [c*512, (c+1)*512).
  - The kernel is HBM-bound: it streams the four weight matrices once, in
    bf16 (halves fp32 traffic; measured rel-err 3.3e-3 vs the 2e-2 budget;
    fp8-e4m3 for ow+y was measured at 3.4e-2 — over budget — so bf16 is
    the floor). All elementwise WKV math stays fp32, and the PE-reshape
    trick stays fp32r so kk keeps full precision into exp(). Total
    stream: 16 MB/core.
  - Phase 1: kk/vv/rr shards: kw[sl,:] @ xk etc. Weights are host-transposed
    so the contraction dim lands on SBUF partitions; the x-vector column is
    the (tiny) stationary operand, weight tiles stream as the moving operand
    at N=512 (1 cycle/row on the PE).
  - WKV recurrence: purely elementwise on the 512-channel shard, done in a
    [128, 4] layout (channel = j*128 + p).
  - Phase 2: partial out = ow[:, sl] @ (r*wkv): 4 k-tiles x 8 n-banks of
    matmuls into a [1, 4096] PSUM row; host sums the 8 per-core partials
    (the "all-reduce" of the column-sharded matvec).
  - new_state = x exactly (token shift), assembled on host.

Stream order is MATRIX-MAJOR (all of rw, then kw, then vw, then ow) so the
dependent chain drains while later weights stream: rr is complete early
(sigmoid runs there), kk next (the kk-only part of the WKV recurrence runs
there), vv third (the short vv tail + y = r*wkv run there, just before the
first ow chunk lands), and the phase-2 matmuls then chase the ow chunks as
they arrive. Only the last ow half-chunk's four matmuls + PSUM copies +
output DMA remain after the final weight byte.

DMA chunks are 1 MB (8 KB per-partition rows; a DMA-only probe measured
4 KB rows ~27% slower and, counterintuitively, 16/32 KB rows ~20% slower
still — 8 KB is the sweet spot). Only ow's last k-tile is split into
512 KB halves, where the arrival quantum sits on the critical tail. The
output DMAs ride the sync HWDGE ring (idle once the weight stream ends;
measured ~1 us faster than a SWDGE descriptor write on the tail).

The PE runs every matmul at ap_size cycles (216 ns at N=512) regardless of
the 1-wide stationary, so phase 1+2 carry ~28 us of mandatory PE time that
must hide under the ~44 us stream. The PE also stalls ~6-8 us in a p-state
transition after its first matmul, so a tiny f32 matmul fires the
transition at ~6 us and the stall completes as the first chunk lands.

k-index convention (phase 1): k = p*32 + n  (p = partition, n = k-tile id),
so W.T.reshape(128, 32, 512) puts k-tile n at [:, n, :] with contraction on
partitions, and x.reshape(128, 32) column n is the matching stationary vec.

The [1,512] -> [128,4] reshape of the phase-1 results runs on the PE: a
matmul whose stationary operand is a zero-padded [128,128] slice with the
data in partition 0 and whose moving operand is the unit vector e0 emits
the row slice as a [128,2] PSUM column pair (channel = j*128 + p). Phase 2
contracts k-tile tt=j over rows [j*128,(j+1)*128) of ow[:, sl].T, matching
that layout.
"""

import ml_dtypes
import numpy as np

import concourse.bass as bass
import concourse.mybir as mybir
import concourse.tile as tile
from concourse import bacc, bass_utils

C = 4096
NCORES = 8
S = C // NCORES          # 512 channels per core
P = 128
KT = C // P              # 32 k-tiles in phase 1
KSUB = 8                 # k-tiles per 1 MB bf16 chunk
OW_KT = S // P           # 4 k-tiles in phase 2
OW_HALF = C // 2         # ow k-tile chunks split into 512 KB column halves
CHUNKS_STD = [(0, 8), (8, 8), (16, 8), (24, 8)]

F32 = mybir.dt.float32
F32R = mybir.dt.float32r
BF16 = mybir.dt.bfloat16
AF = mybir.ActivationFunctionType
NP_BF16 = ml_dtypes.bfloat16

# xvecs layout: [128, 96] = xk[0:32] | xv[32:64] | xr[64:96] (bf16)
# e0 (fp32r) is a separate [128, 2] input: unit vector [1,0,...,0] + a zero
# column (N=2 moving operand of the PE-reshape matmuls — fp32r matmuls
# require an even moving free size)
# svecs layout: [128, 20] = aa[0:4] | bb[4:8] | pp[8:12] | tf[12:16] | td[16:20]
XVECS_W = 3 * KT
SVECS_W = 5 * 4
OFF_XK, OFF_XV, OFF_XR = 0, KT, 2 * KT
OFF_AA, OFF_BB, OFF_PP, OFF_TF, OFF_TD = 0, 4, 8, 12, 16


def _build():
    nc = bacc.Bacc("TRN2", target_bir_lowering=False, debug=False,
                   num_devices=NCORES)

    xvecs_d = nc.dram_tensor("xvecs", [P, XVECS_W], BF16, kind="ExternalInput")
    e0_d = nc.dram_tensor("e0", [P, 2], F32R, kind="ExternalInput")
    svecs_d = nc.dram_tensor("svecs", [P, SVECS_W], F32, kind="ExternalInput")
    wr_d = nc.dram_tensor("wr", [P, KT, S], BF16, kind="ExternalInput")
    wk_d = nc.dram_tensor("wk", [P, KT, S], BF16, kind="ExternalInput")
    wv_d = nc.dram_tensor("wv", [P, KT, S], BF16, kind="ExternalInput")
    wo_d = nc.dram_tensor("wo", [OW_KT, P, C], BF16, kind="ExternalInput")

    partial_d = nc.dram_tensor("partial", [1, C], F32, kind="ExternalOutput")
    nst_d = nc.dram_tensor("nst", [P, 12], F32, kind="ExternalOutput")

    with tile.TileContext(nc) as tc:
        with (
            tc.tile_pool(name="const", bufs=1) as const,
            tc.tile_pool(name="w", bufs=12) as wpool,
            tc.tile_pool(name="ow", bufs=3) as opool,
            tc.tile_pool(name="owh", bufs=2) as ohpool,
            tc.tile_pool(name="small", bufs=1) as small,
        ):
            # z2 feeds the ramp-trigger matmul (emitted once the PSUM pool
            # opens below); its memset leads the gpsimd queue so the trigger
            # can fire as early as possible.
            z2 = small.tile([P, 2], F32)
            nc.gpsimd.memset(z2[:], 0.0)

            # HBM/DMA warm-up: the first ~150 stream descriptors run ~20%
            # slower on a cold memory path (470 vs 387 ns per 8 KB row).
            # A small dummy read with NO consumers, issued first on the
            # gpsimd ring, warms the path before the weight stream starts
            # at ~8.3 us. Nothing waits on it, so it cannot race.
            scratch = const.tile([P, 2, S], BF16)
            nc.gpsimd.dma_start(scratch[:], wr_d[:, 8:10, :])

            # Small INPUT DMAs ride the SWDGE (gpsimd) ring so the SP HWDGE
            # ring carries only weight traffic during the stream (keeps its
            # DMA-completion semaphore lanes clean); the output DMAs ride
            # the then-idle HWDGE ring instead. Emitted BEFORE the big
            # stage_z memset so the descriptor writes are not delayed.
            xvecs = const.tile([P, XVECS_W], BF16)
            nc.gpsimd.dma_start(xvecs[:], xvecs_d[:])
            # pinned weight tile for bridging/filler matmuls
            filler = const.tile([P, 1, S], BF16)
            nc.gpsimd.dma_start(filler[:], wr_d[:, 0:1, :])
            e0 = const.tile([P, 2], F32R)
            nc.gpsimd.dma_start(e0[:], e0_d[:])
            svecs = const.tile([P, SVECS_W], F32)
            nc.gpsimd.dma_start(svecs[:], svecs_d[:])

            # preload the ACT exp LUT off the critical path (the only ACT
            # table the kernel uses: sigmoid is computed via exp+reciprocal)
            warm = small.tile([1, 4], F32)
            nc.gpsimd.memset(warm[:], 0.0)
            warm2 = small.tile([1, 4], F32)
            nc.scalar.activation(warm2[:], warm[:], AF.Exp)

            # stage for the PE reshape: partition 0 carries the phase-1
            # results, rows 1-127 must be finite (they multiply e0's zeros).
            # f32r memset is invalid ISA, so zero an f32 twin and cast-copy.
            stage_z = small.tile([P, 3 * S], F32)
            nc.gpsimd.memset(stage_z[:], 0.0)
            stage = small.tile([P, 3 * S], F32R)
            nc.vector.tensor_copy(stage[:], stage_z[:])

            aa = svecs[:, OFF_AA:OFF_AA + 4]
            bb = svecs[:, OFF_BB:OFF_BB + 4]
            pp = svecs[:, OFF_PP:OFF_PP + 4]
            tf = svecs[:, OFF_TF:OFF_TF + 4]
            td = svecs[:, OFF_TD:OFF_TD + 4]

            def t4(name):
                return small.tile([P, 4], F32, name=name)

            nst = small.tile([P, 12], F32)
            na, nb_t, p2 = nst[:, 0:4], nst[:, 4:8], nst[:, 8:12]
            rkv = small.tile([P, 12], F32)
            rr128 = rkv[:, 0:4]
            kk = rkv[:, 4:8]
            vv = rkv[:, 8:12]

            # ---- phase 1 + overlapped recurrence ---------------------------
            wdrams = [wr_d, wk_d, wv_d]
            xoffs = [OFF_XR, OFF_XK, OFF_XV]
            # WKV temporaries shared between the kk-stage and the vv-stage.
            # y = r*(e1a*aa + e2a*vv)/b is regrouped as y = u + w*vv with
            # u = r*binv*e1a*aa and w = r*binv*e2a, both computable in the
            # kk-stage, so only TWO vector ops + the bf16 cast separate the
            # vv reshape from the phase-2 matmuls.
            r128 = small.tile([P, 4], F32, name="r128")
            e2b = t4("e2b")
            u_t = t4("u_t")
            w_t = t4("w_t")
            y = t4("y")
            y_r = small.tile([P, 4], BF16, name="y_r")

            with tc.tile_pool(name="ps1", bufs=1, space="PSUM") as ps1:
                psums = [ps1.tile([1, S], F32, name=f"ps_{i}") for i in range(3)]
                fl_ps = ps1.tile([1, S], F32, name="fl_ps")
                rs_ps = ps1.tile([P, 24], F32, name="rs_ps")

                # ramp trigger: the PE stalls ~6-8us in a p-state transition
                # after its FIRST matmul, then runs at half speed for ~3us.
                # Fire a tiny f32 matmul as early as possible (the z2 memset
                # is its only dependency) so the stall completes right as the
                # first weight chunk lands.
                zps = ps1.tile([2, 2], F32, name="zps")
                nc.tensor.matmul(zps[:], lhsT=z2[:], rhs=z2[:], start=True, stop=True)

                def fill_mm(n):
                    for i in range(n):
                        nc.tensor.matmul(
                            fl_ps[:],
                            lhsT=xvecs[:, 0:1],
                            rhs=filler[:, 0, :],
                            start=True,
                            stop=True,
                        )

                # bridge the PE from the ramp trigger to the first chunk
                fill_mm(2)
                for wi in range(3):
                    for (kt0, nkt) in CHUNKS_STD:
                        wt = wpool.tile([P, KSUB, S], BF16, tag="wchunk")
                        nc.sync.dma_start(
                            wt[:], wdrams[wi][:, kt0:kt0 + nkt, :])
                        for tl in range(nkt):
                            kt = kt0 + tl
                            nc.tensor.matmul(
                                psums[wi][:],
                                lhsT=xvecs[:, xoffs[wi] + kt:xoffs[wi] + kt + 1],
                                rhs=wt[:, tl, :],
                                start=(kt == 0),
                                stop=(kt == KT - 1),
                            )

                    # matrix wi fully reduced: transpose its [1,512] row into
                    # the [128,4] WKV layout while the next matrix streams
                    if wi == 0:
                        nc.scalar.copy(stage[0:1, 0:S], psums[0][:])
                    else:
                        nc.vector.tensor_copy(
                            stage[0:1, wi * S:(wi + 1) * S], psums[wi][:])
                    if wi == 2:
                        # the PE otherwise idles ~2us through the vv CAST
                        # roundtrip + vector tail and deramps to half clock,
                        # slowing the first ~14 phase-2 matmuls; these
                        # fillers keep it continuously busy through that
                        # window so phase 2 starts at full speed
                        fill_mm(3)
                    for j in range(OW_KT):
                        c2 = 2 * (wi * 4 + j)
                        nc.tensor.matmul(
                            rs_ps[:, c2:c2 + 2],
                            lhsT=stage[:, wi * S + j * P:wi * S + (j + 1) * P],
                            rhs=e0[:],
                            start=True,
                            stop=True,
                        )
                    if wi == 2:
                        fill_mm(4)
                    nc.vector.tensor_copy(
                        rkv[:, wi * 4:wi * 4 + 4],
                        rs_ps[:, 2 * wi * 4:2 * wi * 4 + 8:2])

                    if wi == 0:
                        # r = sigmoid(rr) = 1 / (1 + exp(-rr)) — exp only
                        er = t4("er")
                        nc.scalar.activation(er[:], rr128, AF.Exp, scale=-1.0)
                        rp1 = t4("rp1")
                        nc.vector.tensor_scalar_add(rp1[:], er[:], 1.0)
                        nc.vector.reciprocal(r128[:], rp1[:])
                    elif wi == 1:
                        # everything in the WKV recurrence that needs only kk
                        ww1 = t4("ww1")
                        nc.vector.tensor_add(ww1, tf, kk[:])
                        p1 = t4("p1")
                        nc.vector.tensor_max(p1, pp, ww1)
                        d1 = t4("d1")
                        nc.vector.tensor_sub(d1, pp, p1)
                        e1a = t4("e1a")
                        nc.scalar.activation(e1a, d1, AF.Exp)
                        d2 = t4("d2")
                        nc.vector.tensor_sub(d2, ww1, p1)
                        e2a = t4("e2a")
                        nc.scalar.activation(e2a[:], d2, AF.Exp)
                        acc_a = t4("acc_a")
                        nc.vector.tensor_mul(acc_a[:], e1a, aa)   # e1*aa
                        acc_b = t4("acc_b")
                        nc.vector.tensor_mul(acc_b, e1a, bb)
                        nc.vector.tensor_add(acc_b, acc_b, e2a[:])
                        binv = t4("binv")
                        nc.vector.reciprocal(binv[:], acc_b)
                        # pre-fold r/b into the two y terms: y = u + w*vv
                        rb = t4("rb")
                        nc.vector.tensor_mul(rb, r128[:], binv[:])
                        nc.vector.tensor_mul(u_t[:], acc_a[:], rb)
                        nc.vector.tensor_mul(w_t[:], e2a[:], rb)
                        # state update, kk-only part
                        ww2 = t4("ww2")
                        nc.vector.tensor_add(ww2, pp, td)
                        nc.vector.tensor_max(p2, ww2, kk[:])
                        d3 = t4("d3")
                        nc.vector.tensor_sub(d3, ww2, p2)
                        e1b = t4("e1b")
                        nc.scalar.activation(e1b, d3, AF.Exp)
                        d4 = t4("d4")
                        nc.vector.tensor_sub(d4, kk[:], p2)
                        nc.scalar.activation(e2b[:], d4, AF.Exp)
                        nc.vector.tensor_mul(na, e1b, aa)         # e1*aa
                        nc.vector.tensor_mul(nb_t, e1b, bb)
                        nc.vector.tensor_add(nb_t, nb_t, e2b[:])
                    else:
                        # minimal vv tail on the y-critical path: y = u + w*vv
                        # then the bf16 cast, BEFORE the off-path state update
                        nc.vector.tensor_mul(y[:], w_t[:], vv[:])
                        nc.vector.tensor_add(y[:], u_t[:], y[:])
                        nc.vector.tensor_copy(y_r[:], y[:])
                        tmp_b = t4("tmp_b")
                        nc.vector.tensor_mul(tmp_b, e2b[:], vv[:])
                        nc.vector.tensor_add(na, na, tmp_b)

                # ow weight DMAs: the in-order HWDGE ring plays them right
                # after the phase-1 weight DMAs, by which time y is ready
                # and the phase-2 matmuls chase the arriving chunks. k-tiles
                # 0-2 stream as full 1 MB chunks (8 KB per-partition rows —
                # a DMA-only probe measured 4 KB rows ~27% slower, costing
                # ~2.6 us over ow's 4 MB); only k-tile 3 is split into
                # 512 KB halves to keep the after-last-byte quantum small.
                ofull = []
                for tt in range(OW_KT - 1):
                    ot = opool.tile([P, C], BF16, tag="owfull")
                    nc.sync.dma_start(ot[:], wo_d[tt][:, :])
                    ofull.append(ot)
                ohalf = []
                for half in range(2):
                    ot = ohpool.tile([P, OW_HALF], BF16, tag="owhalf")
                    nc.sync.dma_start(
                        ot[:],
                        wo_d[OW_KT - 1][:, half * OW_HALF:(half + 1) * OW_HALF])
                    ohalf.append(ot)

            nc.gpsimd.dma_start(nst_d[:], nst[:])

            # ---- phase 2: partial = ow[:, sl] @ y ---------------------------
            with tc.tile_pool(name="ps2", bufs=1, space="PSUM") as ps2:
                ow_ps = ps2.tile([1, C], F32)
                out_sb = small.tile([1, C], F32)
                for tt in range(OW_KT):
                    for nb in range(C // 512):
                        if tt < OW_KT - 1:
                            rhs = ofull[tt][:, nb * 512:(nb + 1) * 512]
                        else:
                            half, col = nb // 4, nb % 4
                            rhs = ohalf[half][:, col * 512:(col + 1) * 512]
                        nc.tensor.matmul(
                            ow_ps[:, nb * 512:(nb + 1) * 512],
                            lhsT=y_r[:, tt:tt + 1],
                            rhs=rhs,
                            start=(tt == 0),
                            stop=(tt == OW_KT - 1),
                        )
                        if tt == OW_KT - 1:
                            # bank nb is complete; copy out while later banks
                            # are still accumulating
                            sl_ = slice(nb * 512, (nb + 1) * 512)
                            if nb % 2 == 0:
                                nc.vector.tensor_copy(
                                    out_sb[:, sl_], ow_ps[:, sl_])
                            else:
                                nc.scalar.copy(out_sb[:, sl_], ow_ps[:, sl_])
                            if nb == 3:
                                # first half of the output leaves while banks
                                # 4-7 are still accumulating
                                nc.sync.dma_start(
                                    partial_d[:, 0:C // 2], out_sb[:, 0:C // 2])

            nc.sync.dma_start(partial_d[:, C // 2:], out_sb[:, C // 2:])

    nc.compile()
    return nc


def _prep_in_maps(x, state, state_a, state_b, state_p,
                  time_mix_k, time_mix_v, time_mix_r,
                  time_first, time_decay, kw, vw, rw, ow):
    f = lambda a: np.ascontiguousarray(np.asarray(a), dtype=np.float32)
    x, state = f(x), f(state)
    tmk, tmv, tmr = f(time_mix_k), f(time_mix_v), f(time_mix_r)
    xk = (x * tmk + state * (1.0 - tmk)).reshape(P, KT)
    xv = (x * tmv + state * (1.0 - tmv)).reshape(P, KT)
    xr = (x * tmr + state * (1.0 - tmr)).reshape(P, KT)
    aa, bb, pp = f(state_a), f(state_b), f(state_p)
    tf, td = f(time_first), f(time_decay)
    kw, vw, rw, ow = f(kw), f(vw), f(rw), f(ow)

    xvecs = np.zeros((P, XVECS_W), dtype=np.float32)
    xvecs[:, OFF_XK:OFF_XK + KT] = xk
    xvecs[:, OFF_XV:OFF_XV + KT] = xv
    xvecs[:, OFF_XR:OFF_XR + KT] = xr
    xvecs = xvecs.astype(NP_BF16)

    e0 = np.zeros((P, 2), dtype=np.float32)
    e0[0, 0] = 1.0

    wb = lambda a: np.ascontiguousarray(a).astype(NP_BF16)

    # WKV-side [128, 4] layout: channel = j*128 + p
    pm = lambda v: np.ascontiguousarray(v.reshape(OW_KT, P).T)
    in_maps = []
    for c in range(NCORES):
        sl = slice(c * S, (c + 1) * S)
        svecs = np.empty((P, SVECS_W), dtype=np.float32)
        svecs[:, OFF_AA:OFF_AA + 4] = pm(aa[sl])
        svecs[:, OFF_BB:OFF_BB + 4] = pm(bb[sl])
        svecs[:, OFF_PP:OFF_PP + 4] = pm(pp[sl])
        svecs[:, OFF_TF:OFF_TF + 4] = pm(tf[sl])
        svecs[:, OFF_TD:OFF_TD + 4] = pm(td[sl])
        in_maps.append({
            "xvecs": xvecs,
            "e0": e0,
            "svecs": svecs,
            "wr": wb(rw[sl, :].T).reshape(P, KT, S),
            "wk": wb(kw[sl, :].T).reshape(P, KT, S),
            "wv": wb(vw[sl, :].T).reshape(P, KT, S),
            "wo": wb(ow[:, sl].T).reshape(OW_KT, P, C),
        })
    return in_maps, x


_NC_CACHE = None


def _run(inputs, trace=False):
    global _NC_CACHE
    if _NC_CACHE is None:
        _NC_CACHE = _build()
    nc = _NC_CACHE
    in_maps, x = _prep_in_maps(**inputs)
    res = bass_utils.run_bass_kernel_spmd(
        nc, in_maps, core_ids=list(range(NCORES)), trace=trace)

    out = np.zeros(C, dtype=np.float32)
    new_a = np.empty(C, dtype=np.float32)
    new_b = np.empty(C, dtype=np.float32)
    new_p = np.empty(C, dtype=np.float32)
    for c in range(NCORES):
        r = res.results[c]
        out += r["partial"].reshape(C)
        sl = slice(c * S, (c + 1) * S)
        nst = r["nst"]
        # [p, j] -> channel j*128 + p
        new_a[sl] = nst[:, 0:4].T.reshape(S)
        new_b[sl] = nst[:, 4:8].T.reshape(S)
        new_p[sl] = nst[:, 8:12].T.reshape(S)
    return (out, x.copy(), new_a, new_b, new_p), res


def kernel(**inputs):
    outs, _ = _run(inputs, trace=False)
    return outs


# revision 56
# speedup vs baseline: 1.1798x; 1.1228x over previous
"""RWKV time-mixing (C=4096) on 8 trn2 NeuronCores.

Strategy (tensor-parallel over channels, M=8 cores, S=C/M=512):
  - Core c owns channels sl = [c*512, (c+1)*512).
  - The kernel is HBM-bound: it streams the four weight matrices once, in
    bf16 (halves fp32 traffic; measured rel-err 3.3e-3 vs the 2e-2 budget;
    fp8-e4m3 for ow+y was measured at 3.4e-2 — over budget — so bf16 is
    the floor). All elementwise WKV math stays fp32, and the PE-reshape
    trick stays fp32r so kk keeps full precision into exp(). Total
    stream: 16 MB/core.
  - Phase 1: kk/vv/rr shards: kw[sl,:] @ xk etc. Weights are host-transposed
    so the contraction dim lands on SBUF partitions; the x-vector column is
    the (tiny) stationary operand, weight tiles stream as the moving operand
    at N=512 (1 cycle/row on the PE).
  - WKV recurrence: purely elementwise on the 512-channel shard, done in a
    [128, 4] layout (channel = j*128 + p).
  - Phase 2: partial out = ow[:, sl] @ (r*wkv): 4 k-tiles x 8 n-banks of
    matmuls into a [1, 4096] PSUM row; host sums the 8 per-core partials
    (the "all-reduce" of the column-sharded matvec).
  - new_state = x exactly (token shift), assembled on host.

Stream order is MATRIX-MAJOR (all of rw, then kw, then vw, then ow) so the
dependent chain drains while later weights stream: rr is complete early
(sigmoid runs there), kk next (the kk-only part of the WKV recurrence runs
there), vv third (the short vv tail + y = r*wkv run there, just before the
first ow chunk lands), and the phase-2 matmuls then chase the ow chunks as
they arrive. Only the last ow half-chunk's four matmuls + PSUM copies +
output DMA remain after the final weight byte.

DMA chunks are 1 MB (8 KB per-partition rows; a DMA-only probe measured
4 KB rows ~27% slower and, counterintuitively, 16/32 KB rows ~20% slower
still — 8 KB is the sweet spot). Only ow's last k-tile is split into
512 KB halves, where the arrival quantum sits on the critical tail. The
output DMAs ride the sync HWDGE ring (idle once the weight stream ends;
measured ~1 us faster than a SWDGE descriptor write on the tail).

The PE runs every matmul at ap_size cycles (216 ns at N=512) regardless of
the 1-wide stationary, so phase 1+2 carry ~28 us of mandatory PE time that
must hide under the ~44 us stream. The PE also stalls ~6-8 us in a p-state
transition after its first matmul, so a tiny f32 matmul fires the
transition at ~6 us and the stall completes as the first chunk lands.

k-index convention (phase 1): k = p*32 + n  (p = partition, n = k-tile id),
so W.T.reshape(128, 32, 512) puts k-tile n at [:, n, :] with contraction on
partitions, and x.reshape(128, 32) column n is the matching stationary vec.

The [1,512] -> [128,4] reshape of the phase-1 results runs on the PE: a
matmul whose stationary operand is a zero-padded [128,128] slice with the
data in partition 0 and whose moving operand is the unit vector e0 emits
the row slice as a [128,2] PSUM column pair (channel = j*128 + p). Phase 2
contracts k-tile tt=j over rows [j*128,(j+1)*128) of ow[:, sl].T, matching
that layout.
"""

import ml_dtypes
import numpy as np

import concourse.bass as bass
import concourse.mybir as mybir
import concourse.tile as tile
from concourse import bacc, bass_utils

C = 4096
NCORES = 8
S = C // NCORES          # 512 channels per core
P = 128
KT = C // P              # 32 k-tiles in phase 1
KSUB = 8                 # k-tiles per 1 MB bf16 chunk
OW_KT = S // P           # 4 k-tiles in phase 2
OW_HALF = C // 2         # ow k-tile chunks split into 512 KB column halves
CHUNKS_STD = [(0, 8), (8, 8), (16, 8), (24, 8)]

F32 = mybir.dt.float32
F32R = mybir.dt.float32r
BF16 = mybir.dt.bfloat16
AF = mybir.ActivationFunctionType
NP_BF16 = ml_dtypes.bfloat16

# xvecs layout: [128, 96] = xk[0:32] | xv[32:64] | xr[64:96] (bf16)
# e0 (fp32r) is a separate [128, 2] input: unit vector [1,0,...,0] + a zero
# column (N=2 moving operand of the PE-reshape matmuls — fp32r matmuls
# require an even moving free size)
# svecs layout: [128, 20] = aa[0:4] | bb[4:8] | pp[8:12] | tf[12:16] | td[16:20]
XVECS_W = 3 * KT
SVECS_W = 5 * 4
OFF_XK, OFF_XV, OFF_XR = 0, KT, 2 * KT
OFF_AA, OFF_BB, OFF_PP, OFF_TF, OFF_TD = 0, 4, 8, 12, 16


def _build():
    nc = bacc.Bacc("TRN2", target_bir_lowering=False, debug=False,
                   num_devices=NCORES)

    xvecs_d = nc.dram_tensor("xvecs", [P, XVECS_W], BF16, kind="ExternalInput")
    e0_d = nc.dram_tensor("e0", [P, 2], F32R, kind="ExternalInput")
    svecs_d = nc.dram_tensor("svecs", [P, SVECS_W], F32, kind="ExternalInput")
    wr_d = nc.dram_tensor("wr", [P, KT, S], BF16, kind="ExternalInput")
    wk_d = nc.dram_tensor("wk", [P, KT, S], BF16, kind="ExternalInput")
    wv_d = nc.dram_tensor("wv", [P, KT, S], BF16, kind="ExternalInput")
    wo_d = nc.dram_tensor("wo", [OW_KT, P, C], BF16, kind="ExternalInput")

    partial_d = nc.dram_tensor("partial", [1, C], F32, kind="ExternalOutput")
    nst_d = nc.dram_tensor("nst", [P, 12], F32, kind="ExternalOutput")

    with tile.TileContext(nc) as tc:
        with (
            tc.tile_pool(name="const", bufs=1) as const,
            tc.tile_pool(name="w", bufs=12) as wpool,
            tc.tile_pool(name="ow", bufs=3) as opool,
            tc.tile_pool(name="owh", bufs=2) as ohpool,
            tc.tile_pool(name="small", bufs=1) as small,
        ):
            # z2 feeds the ramp-trigger matmul (emitted once the PSUM pool
            # opens below); its memset leads the gpsimd queue so the trigger
            # can fire as early as possible.
            z2 = small.tile([P, 2], F32)
            nc.gpsimd.memset(z2[:], 0.0)

            # Small INPUT DMAs ride the SWDGE (gpsimd) ring so the SP HWDGE
            # ring carries only weight traffic during the stream (keeps its
            # DMA-completion semaphore lanes clean); the output DMAs ride
            # the then-idle HWDGE ring instead. Emitted BEFORE the big
            # stage_z memset so the descriptor writes are not delayed.
            xvecs = const.tile([P, XVECS_W], BF16)
            nc.gpsimd.dma_start(xvecs[:], xvecs_d[:])
            # pinned weight tile for bridging/filler matmuls
            filler = const.tile([P, 1, S], BF16)
            nc.gpsimd.dma_start(filler[:], wr_d[:, 0:1, :])
            e0 = const.tile([P, 2], F32R)
            nc.gpsimd.dma_start(e0[:], e0_d[:])
            svecs = const.tile([P, SVECS_W], F32)
            nc.gpsimd.dma_start(svecs[:], svecs_d[:])

            # preload the ACT exp LUT off the critical path (the only ACT
            # table the kernel uses: sigmoid is computed via exp+reciprocal)
            warm = small.tile([1, 4], F32)
            nc.gpsimd.memset(warm[:], 0.0)
            warm2 = small.tile([1, 4], F32)
            nc.scalar.activation(warm2[:], warm[:], AF.Exp)

            # stage for the PE reshape: partition 0 carries the phase-1
            # results, rows 1-127 must be finite (they multiply e0's zeros).
            # f32r memset is invalid ISA, so zero an f32 twin and cast-copy.
            stage_z = small.tile([P, 3 * S], F32)
            nc.gpsimd.memset(stage_z[:], 0.0)
            stage = small.tile([P, 3 * S], F32R)
            nc.vector.tensor_copy(stage[:], stage_z[:])

            aa = svecs[:, OFF_AA:OFF_AA + 4]
            bb = svecs[:, OFF_BB:OFF_BB + 4]
            pp = svecs[:, OFF_PP:OFF_PP + 4]
            tf = svecs[:, OFF_TF:OFF_TF + 4]
            td = svecs[:, OFF_TD:OFF_TD + 4]

            def t4(name):
                return small.tile([P, 4], F32, name=name)

            nst = small.tile([P, 12], F32)
            na, nb_t, p2 = nst[:, 0:4], nst[:, 4:8], nst[:, 8:12]
            rkv = small.tile([P, 12], F32)
            rr128 = rkv[:, 0:4]
            kk = rkv[:, 4:8]
            vv = rkv[:, 8:12]

            # ---- phase 1 + overlapped recurrence ---------------------------
            wdrams = [wr_d, wk_d, wv_d]
            xoffs = [OFF_XR, OFF_XK, OFF_XV]
            # WKV temporaries shared between the kk-stage and the vv-stage.
            # y = r*(e1a*aa + e2a*vv)/b is regrouped as y = u + w*vv with
            # u = r*binv*e1a*aa and w = r*binv*e2a, both computable in the
            # kk-stage, so only TWO vector ops + the bf16 cast separate the
            # vv reshape from the phase-2 matmuls.
            r128 = small.tile([P, 4], F32, name="r128")
            e2b = t4("e2b")
            u_t = t4("u_t")
            w_t = t4("w_t")
            y = t4("y")
            y_r = small.tile([P, 4], BF16, name="y_r")

            with tc.tile_pool(name="ps1", bufs=1, space="PSUM") as ps1:
                psums = [ps1.tile([1, S], F32, name=f"ps_{i}") for i in range(3)]
                fl_ps = ps1.tile([1, S], F32, name="fl_ps")
                rs_ps = ps1.tile([P, 24], F32, name="rs_ps")

                # ramp trigger: the PE stalls ~6-8us in a p-state transition
                # after its FIRST matmul, then runs at half speed for ~3us.
                # Fire a tiny f32 matmul as early as possible (the z2 memset
                # is its only dependency) so the stall completes right as the
                # first weight chunk lands.
                zps = ps1.tile([2, 2], F32, name="zps")
                nc.tensor.matmul(zps[:], lhsT=z2[:], rhs=z2[:], start=True, stop=True)

                def fill_mm(n):
                    for i in range(n):
                        nc.tensor.matmul(
                            fl_ps[:],
                            lhsT=xvecs[:, 0:1],
                            rhs=filler[:, 0, :],
                            start=True,
                            stop=True,
                        )

                # bridge the PE from the ramp trigger to the first chunk
                fill_mm(2)
                for wi in range(3):
                    for (kt0, nkt) in CHUNKS_STD:
                        wt = wpool.tile([P, KSUB, S], BF16, tag="wchunk")
                        nc.sync.dma_start(
                            wt[:], wdrams[wi][:, kt0:kt0 + nkt, :])
                        for tl in range(nkt):
                            kt = kt0 + tl
                            nc.tensor.matmul(
                                psums[wi][:],
                                lhsT=xvecs[:, xoffs[wi] + kt:xoffs[wi] + kt + 1],
                                rhs=wt[:, tl, :],
                                start=(kt == 0),
                                stop=(kt == KT - 1),
                            )

                    # matrix wi fully reduced: transpose its [1,512] row into
                    # the [128,4] WKV layout while the next matrix streams
                    if wi == 0:
                        nc.scalar.copy(stage[0:1, 0:S], psums[0][:])
                    else:
                        nc.vector.tensor_copy(
                            stage[0:1, wi * S:(wi + 1) * S], psums[wi][:])
                    if wi == 2:
                        # the PE otherwise idles ~2us through the vv CAST
                        # roundtrip + vector tail and deramps to half clock,
                        # slowing the first ~14 phase-2 matmuls; these
                        # fillers keep it continuously busy through that
                        # window so phase 2 starts at full speed
                        fill_mm(3)
                    for j in range(OW_KT):
                        c2 = 2 * (wi * 4 + j)
                        nc.tensor.matmul(
                            rs_ps[:, c2:c2 + 2],
                            lhsT=stage[:, wi * S + j * P:wi * S + (j + 1) * P],
                            rhs=e0[:],
                            start=True,
                            stop=True,
                        )
                    if wi == 2:
                        fill_mm(4)
                    nc.vector.tensor_copy(
                        rkv[:, wi * 4:wi * 4 + 4],
                        rs_ps[:, 2 * wi * 4:2 * wi * 4 + 8:2])

                    if wi == 0:
                        # r = sigmoid(rr) = 1 / (1 + exp(-rr)) — exp only
                        er = t4("er")
                        nc.scalar.activation(er[:], rr128, AF.Exp, scale=-1.0)
                        rp1 = t4("rp1")
                        nc.vector.tensor_scalar_add(rp1[:], er[:], 1.0)
                        nc.vector.reciprocal(r128[:], rp1[:])
                    elif wi == 1:
                        # everything in the WKV recurrence that needs only kk
                        ww1 = t4("ww1")
                        nc.vector.tensor_add(ww1, tf, kk[:])
                        p1 = t4("p1")
                        nc.vector.tensor_max(p1, pp, ww1)
                        d1 = t4("d1")
                        nc.vector.tensor_sub(d1, pp, p1)
                        e1a = t4("e1a")
                        nc.scalar.activation(e1a, d1, AF.Exp)
                        d2 = t4("d2")
                        nc.vector.tensor_sub(d2, ww1, p1)
                        e2a = t4("e2a")
                        nc.scalar.activation(e2a[:], d2, AF.Exp)
                        acc_a = t4("acc_a")
                        nc.vector.tensor_mul(acc_a[:], e1a, aa)   # e1*aa
                        acc_b = t4("acc_b")
                        nc.vector.tensor_mul(acc_b, e1a, bb)
                        nc.vector.tensor_add(acc_b, acc_b, e2a[:])
                        binv = t4("binv")
                        nc.vector.reciprocal(binv[:], acc_b)
                        # pre-fold r/b into the two y terms: y = u + w*vv
                        rb = t4("rb")
                        nc.vector.tensor_mul(rb, r128[:], binv[:])
                        nc.vector.tensor_mul(u_t[:], acc_a[:], rb)
                        nc.vector.tensor_mul(w_t[:], e2a[:], rb)
                        # state update, kk-only part
                        ww2 = t4("ww2")
                        nc.vector.tensor_add(ww2, pp, td)
                        nc.vector.tensor_max(p2, ww2, kk[:])
                        d3 = t4("d3")
                        nc.vector.tensor_sub(d3, ww2, p2)
                        e1b = t4("e1b")
                        nc.scalar.activation(e1b, d3, AF.Exp)
                        d4 = t4("d4")
                        nc.vector.tensor_sub(d4, kk[:], p2)
                        nc.scalar.activation(e2b[:], d4, AF.Exp)
                        nc.vector.tensor_mul(na, e1b, aa)         # e1*aa
                        nc.vector.tensor_mul(nb_t, e1b, bb)
                        nc.vector.tensor_add(nb_t, nb_t, e2b[:])
                    else:
                        # minimal vv tail on the y-critical path: y = u + w*vv
                        # then the bf16 cast, BEFORE the off-path state update
                        nc.vector.tensor_mul(y[:], w_t[:], vv[:])
                        nc.vector.tensor_add(y[:], u_t[:], y[:])
                        nc.vector.tensor_copy(y_r[:], y[:])
                        tmp_b = t4("tmp_b")
                        nc.vector.tensor_mul(tmp_b, e2b[:], vv[:])
                        nc.vector.tensor_add(na, na, tmp_b)

                # ow weight DMAs: the in-order HWDGE ring plays them right
                # after the phase-1 weight DMAs, by which time y is ready
                # and the phase-2 matmuls chase the arriving chunks. k-tiles
                # 0-2 stream as full 1 MB chunks (8 KB per-partition rows —
                # a DMA-only probe measured 4 KB rows ~27% slower, costing
                # ~2.6 us over ow's 4 MB); only k-tile 3 is split into
                # 512 KB halves to keep the after-last-byte quantum small.
                ofull = []
                for tt in range(OW_KT - 1):
                    ot = opool.tile([P, C], BF16, tag="owfull")
                    nc.sync.dma_start(ot[:], wo_d[tt][:, :])
                    ofull.append(ot)
                ohalf = []
                for half in range(2):
                    ot = ohpool.tile([P, OW_HALF], BF16, tag="owhalf")
                    nc.sync.dma_start(
                        ot[:],
                        wo_d[OW_KT - 1][:, half * OW_HALF:(half + 1) * OW_HALF])
                    ohalf.append(ot)

            nc.gpsimd.dma_start(nst_d[:], nst[:])

            # ---- phase 2: partial = ow[:, sl] @ y ---------------------------
            with tc.tile_pool(name="ps2", bufs=1, space="PSUM") as ps2:
                ow_ps = ps2.tile([1, C], F32)
                out_sb = small.tile([1, C], F32)
                for tt in range(OW_KT):
                    for nb in range(C // 512):
                        if tt < OW_KT - 1:
                            rhs = ofull[tt][:, nb * 512:(nb + 1) * 512]
                        else:
                            half, col = nb // 4, nb % 4
                            rhs = ohalf[half][:, col * 512:(col + 1) * 512]
                        nc.tensor.matmul(
                            ow_ps[:, nb * 512:(nb + 1) * 512],
                            lhsT=y_r[:, tt:tt + 1],
                            rhs=rhs,
                            start=(tt == 0),
                            stop=(tt == OW_KT - 1),
                        )
                        if tt == OW_KT - 1:
                            # bank nb is complete; copy out while later banks
                            # are still accumulating
                            sl_ = slice(nb * 512, (nb + 1) * 512)
                            if nb % 2 == 0:
                                nc.vector.tensor_copy(
                                    out_sb[:, sl_], ow_ps[:, sl_])
                            else:
                                nc.scalar.copy(out_sb[:, sl_], ow_ps[:, sl_])
                            if nb == 3:
                                # first half of the output leaves while banks
                                # 4-7 are still accumulating
                                nc.sync.dma_start(
                                    partial_d[:, 0:C // 2], out_sb[:, 0:C // 2])

            nc.sync.dma_start(partial_d[:, C // 2:], out_sb[:, C // 2:])

    nc.compile()
    return nc


def _prep_in_maps(x, state, state_a, state_b, state_p,
                  time_mix_k, time_mix_v, time_mix_r,
                  time_first, time_decay, kw, vw, rw, ow):
    f = lambda a: np.ascontiguousarray(np.asarray(a), dtype=np.float32)
    x, state = f(x), f(state)
    tmk, tmv, tmr = f(time_mix_k), f(time_mix_v), f(time_mix_r)
    xk = (x * tmk + state * (1.0 - tmk)).reshape(P, KT)
    xv = (x * tmv + state * (1.0 - tmv)).reshape(P, KT)
    xr = (x * tmr + state * (1.0 - tmr)).reshape(P, KT)
    aa, bb, pp = f(state_a), f(state_b), f(state_p)
    tf, td = f(time_first), f(time_decay)
    kw, vw, rw, ow = f(kw), f(vw), f(rw), f(ow)

    xvecs = np.zeros((P, XVECS_W), dtype=np.float32)
    xvecs[:, OFF_XK:OFF_XK + KT] = xk
    xvecs[:, OFF_XV:OFF_XV + KT] = xv
    xvecs[:, OFF_XR:OFF_XR + KT] = xr
    xvecs = xvecs.astype(NP_BF16)

    e0 = np.zeros((P, 2), dtype=np.float32)
    e0[0, 0] = 1.0

    wb = lambda a: np.ascontiguousarray(a).astype(NP_BF16)

    # WKV-side [128, 4] layout: channel = j*128 + p
    pm = lambda v: np.ascontiguousarray(v.reshape(OW_KT, P).T)
    in_maps = []
    for c in range(NCORES):
        sl = slice(c * S, (c + 1) * S)
        svecs = np.empty((P, SVECS_W), dtype=np.float32)
        svecs[:, OFF_AA:OFF_AA + 4] = pm(aa[sl])
        svecs[:, OFF_BB:OFF_BB + 4] = pm(bb[sl])
        svecs[:, OFF_PP:OFF_PP + 4] = pm(pp[sl])
        svecs[:, OFF_TF:OFF_TF + 4] = pm(tf[sl])
        svecs[:, OFF_TD:OFF_TD + 4] = pm(td[sl])
        in_maps.append({
            "xvecs": xvecs,
            "e0": e0,
            "svecs": svecs,
            "wr": wb(rw[sl, :].T).reshape(P, KT, S),
            "wk": wb(kw[sl, :].T).reshape(P, KT, S),
            "wv": wb(vw[sl, :].T).reshape(P, KT, S),
            "wo": wb(ow[:, sl].T).reshape(OW_KT, P, C),
        })
    return in_maps, x


_NC_CACHE = None


def _run(inputs, trace=False):
    global _NC_CACHE
    if _NC_CACHE is None:
        _NC_CACHE = _build()
    nc = _NC_CACHE
    in_maps, x = _prep_in_maps(**inputs)
    res = bass_utils.run_bass_kernel_spmd(
        nc, in_maps, core_ids=list(range(NCORES)), trace=trace)

    out = np.zeros(C, dtype=np.float32)
    new_a = np.empty(C, dtype=np.float32)
    new_b = np.empty(C, dtype=np.float32)
    new_p = np.empty(C, dtype=np.float32)
    for c in range(NCORES):
        r = res.results[c]
        out += r["partial"].reshape(C)
        sl = slice(c * S, (c + 1) * S)
        nst = r["nst"]
        # [p, j] -> channel j*128 + p
        new_a[sl] = nst[:, 0:4].T.reshape(S)
        new_b[sl] = nst[:, 4:8].T.reshape(S)
        new_p[sl] = nst[:, 8:12].T.reshape(S)
    return (out, x.copy(), new_a, new_b, new_p), res


def kernel(**inputs):
    outs, _ = _run(inputs, trace=False)
    return outs
